# revision 1
# baseline (speedup 1.0000x reference)
"""GIN message-passing (2 GINConv layers + 2 linear) on 8 TRN2 NeuronCores.

Strategy (self-contained; shapes hardcoded for the 100k-node / 1.6M-edge
problem):
  - Shard dst nodes across 8 cores (12500 each). Each core owns the edges
    whose dst is in its shard; dst tiles of 128 nodes.
  - Per tile-batch, gather source-node rows with gpsimd.dma_gather (int16
    indices -> node table split into 4 groups <=32768 rows; <=1024 idx per
    call; calls round-robin over 4 SWDGE queues so all Q7 pairs work), then
    aggregate with one-hot matmuls: aggT[ch, dst] += Xe[slots, ch].T @ onehot.
  - Layer 1 gathers x in fp16 (256B rows, table replicated via inputs),
    computes z = relu((x + A@x)@W1 + b1) @ W2 per shard, writes z rows fp16
    (padded to 128 ch) to HBM; a SLICED AllGather (overlapped with phase 1)
    shares z: z_full layout is [slice][core][row] so each slice's AG output
    is contiguous.
  - Layer 2 gathers z rows (fp16 256B), h2 = relu(z_dst + A@z + b2),
    h3 = relu(h2@W3+b3), out = h3@W4+b4.
  - Per-(tile,group) slot budgets are static (max over cores, x128) with
    idx-0 padding masked by dstloc=-1 one-hots.
"""

import numpy as np

P = 128


class Cfg:
    def __init__(self, n_nodes, n_edges, in_ch, hid, n_cores, t_b, n_slices=4):
        self.N = n_nodes
        self.E = n_edges
        self.CH = in_ch
        self.H = hid
        self.NCORE = n_cores
        self.SHARD = n_nodes // n_cores
        self.NT = -(-self.SHARD // P)
        self.NQ = 4  # x16 table groups (quarters)
        self.QS = n_nodes // 4
        assert self.QS <= 32768
        self.NS = n_slices  # z table groups (AG slices)
        assert self.SHARD % n_slices == 0
        self.SROW = self.SHARD // n_slices  # shard rows per slice
        assert self.SROW * n_cores <= 32768
        self.T_B = t_b
        self.NB = -(-self.NT // t_b)


FULL = Cfg(100000, 1600000, 128, 64, 8, 4)


def _r128(a):
    return ((a + 127) // 128) * 128


class LayerPlan:
    """Gather schedule for one layer: per-(tile, group) budgets, batched
    chunk layout, per-core idx/dstloc arrays."""

    def __init__(self, cfg, n_groups, per_core_edges):
        # per_core_edges: [core] -> (t, grp, lidx, dl) int arrays
        c = cfg
        self.cfg = c
        self.NG = n_groups
        counts = np.zeros((c.NCORE, c.NT, n_groups), dtype=np.int64)
        self.groups = []
        for ci in range(c.NCORE):
            t, grp, lidx, dl = per_core_edges[ci]
            key = t * n_groups + grp
            order = np.argsort(key, kind="stable")
            key = key[order]
            lidx = lidx[order].astype(np.int16)
            dl = dl[order].astype(np.int16)
            cnt = np.bincount(key, minlength=c.NT * n_groups).reshape(
                c.NT, n_groups
            )
            counts[ci] = cnt
            bounds = np.concatenate([[0], np.cumsum(cnt.reshape(-1))])
            g = {}
            for tt in range(c.NT):
                for qq in range(n_groups):
                    k = tt * n_groups + qq
                    lo, hi = bounds[k], bounds[k + 1]
                    if hi > lo:
                        g[(tt, qq)] = (lidx[lo:hi], dl[lo:hi])
            self.groups.append(g)

        # x128 so every tile's segment is chunk-aligned (PE matmuls need
        # base partition 0) and every matmul has full K=128
        self.B = _r128(counts.max(axis=0))  # [NT, NG]
        self.batches = []
        cgo = 0
        igo = 0
        for b in range(c.NB):
            tiles = list(range(b * c.T_B, min((b + 1) * c.T_B, c.NT)))
            quarters = []
            o = 0
            for q in range(n_groups):
                segs = []
                s0 = 0
                for t in tiles:
                    btq = int(self.B[t, q])
                    if btq == 0:
                        continue
                    lo, hi = s0, s0 + btq
                    for ch in range(lo // P, (hi - 1) // P + 1):
                        segs.append((t, ch))
                    s0 = hi
                tot = s0
                nch = tot // P
                quarters.append(
                    dict(tot=tot, nch=nch, o=o, icols=tot // 16, segs=segs)
                )
                o += nch
            self.batches.append(
                dict(tiles=tiles, quarters=quarters, cgo=cgo, igo=igo, nch=o)
            )
            cgo += o
            igo += sum(qq["icols"] for qq in quarters)
        self.CGOT = cgo
        self.IGOT = igo

    def core_arrays(self, ci):
        c = self.cfg
        idx16 = np.zeros((P, self.IGOT), dtype=np.int16)
        dstloc = np.full((P, self.CGOT), -1.0, dtype=np.float16)
        g = self.groups[ci]
        for b in self.batches:
            icol = b["igo"]
            for q, qq in enumerate(b["quarters"]):
                tot = qq["tot"]
                if tot == 0:
                    continue
                sl_full = np.zeros(tot, dtype=np.int16)
                dl_full = np.full(tot, -1, dtype=np.int16)
                s0 = 0
                for t in b["tiles"]:
                    btq = int(self.B[t, q])
                    if btq == 0:
                        continue
                    if (t, q) in g:
                        sl, dl = g[(t, q)]
                        n = len(sl)
                        sl_full[s0 : s0 + n] = sl
                        dl_full[s0 : s0 + n] = dl
                    s0 += btq
                blk = sl_full.reshape(tot // 16, 16).T
                idx16[:, icol : icol + tot // 16] = np.tile(blk, (8, 1))
                icol += tot // 16
                col0 = b["cgo"] + qq["o"]
                nch = qq["nch"]
                dstloc[:, col0 : col0 + nch] = (
                    dl_full.reshape(nch, P).T.astype(np.float16)
                )
        return idx16, dstloc


class Plan:
    def __init__(self, cfg, src, dst):
        c = cfg
        self.cfg = c
        core = dst // c.SHARD
        e1, e2 = [], []
        for ci in range(c.NCORE):
            m = core == ci
            s = src[m]
            d = dst[m] - ci * c.SHARD
            t = d // P
            dl = d % P
            # layer 1: x16 table grouped by quarters of node id
            e1.append((t, s // c.QS, s % c.QS, dl))
            # layer 2: z_full is [slice][core][row]; slice = shard-row/SROW
            sc = s // c.SHARD
            sr = s % c.SHARD
            sl = sr // c.SROW
            lidx = sc * c.SROW + (sr - sl * c.SROW)
            e2.append((t, sl, lidx, dl))
        self.L1 = LayerPlan(cfg, c.NQ, e1)
        self.L2 = LayerPlan(cfg, c.NS, e2)

    def core_arrays(self, ci):
        i1, d1 = self.L1.core_arrays(ci)
        i2, d2 = self.L2.core_arrays(ci)
        return np.concatenate(
            [i1, d1.view(np.int16), i2, d2.view(np.int16)], axis=1
        )


def _build(plan):
    import concourse.tile as tile
    from concourse import bacc, mybir

    c = plan.cfg
    f16, f32, i16 = mybir.dt.float16, mybir.dt.float32, mybir.dt.int16
    CH, H, QS, SHARD = c.CH, c.H, c.QS, c.SHARD
    L1, L2 = plan.L1, plan.L2
    PKW = L1.IGOT + L1.CGOT + L2.IGOT + L2.CGOT

    nc = bacc.Bacc(
        "TRN2", target_bir_lowering=False, debug=False, num_devices=c.NCORE,
        num_swdge_queues=4,
    )
    x16 = nc.dram_tensor("x16", [c.N, CH], f16, kind="ExternalInput")
    xT = nc.dram_tensor("xT", [CH, SHARD], f32, kind="ExternalInput")
    pk = nc.dram_tensor("pk", [P, PKW], i16, kind="ExternalInput")
    iota = nc.dram_tensor("iota", [P, P], f16, kind="ExternalInput")
    ident = nc.dram_tensor("ident", [H, H], f16, kind="ExternalInput")
    w1 = nc.dram_tensor("w1", [CH, H], f16, kind="ExternalInput")
    w2 = nc.dram_tensor("w2", [H, H], f16, kind="ExternalInput")
    w3 = nc.dram_tensor("w3", [H, 16], f16, kind="ExternalInput")
    w4 = nc.dram_tensor("w4", [16, 1], f16, kind="ExternalInput")
    b1 = nc.dram_tensor("b1", [H, 1], f32, kind="ExternalInput")
    b2 = nc.dram_tensor("b2", [H, 1], f32, kind="ExternalInput")
    b3 = nc.dram_tensor("b3", [16, 1], f32, kind="ExternalInput")
    b4v = nc.dram_tensor("b4v", [1, 1], f32, kind="ExternalInput")
    out = nc.dram_tensor("out", [1, SHARD], f32, kind="ExternalOutput")

    with tile.TileContext(nc) as tc:
        with (
            tc.tile_pool(name="const", bufs=1) as cp,
            tc.tile_pool(name="persist", bufs=1) as pp,
            tc.tile_pool(name="dram", bufs=1, space="DRAM") as dp,
        ):
            def load_const(name, t, shape, dt):
                sb = cp.tile(shape, dt, name=name + "_sb")
                nc.sync.dma_start(out=sb[:], in_=t[:, :])
                return sb

            iota_sb = load_const("iota", iota, [P, P], f16)
            id_sb = load_const("id", ident, [H, H], f16)
            w1_sb = load_const("w1", w1, [CH, H], f16)
            w2_sb = load_const("w2", w2, [H, H], f16)
            w3_sb = load_const("w3", w3, [H, 16], f16)
            w4_sb = load_const("w4", w4, [16, 1], f16)
            b1_sb = load_const("b1", b1, [H, 1], f32)
            b2_sb = load_const("b2", b2, [H, 1], f32)
            b3_sb = load_const("b3", b3, [16, 1], f32)
            b4_sb = load_const("b4", b4v, [1, 1], f32)

            zT_sh = pp.tile([H, SHARD], f32)
            outT = pp.tile([1, SHARD], f32)
            z_shard = dp.tile([SHARD, P], f16)
            z_full = dp.tile([c.N, P], f16)  # [slice][core][row][ch]

            relu = mybir.ActivationFunctionType.Relu
            qrr = [0]  # round-robin SWDGE queue
            state = {}

            def tile_cols(t):
                return min(P, SHARD - t * P)

            def gather_batch(lp, b, pk_base, gbuf, table, tbase_of, elem):
                sm = state["sm"]
                icols = sum(q["icols"] for q in b["quarters"])
                nch = b["nch"]
                pk_sb = sm.tile([P, icols + nch], i16, tag="pk")
                nc.sync.dma_start(
                    out=pk_sb[:, :icols],
                    in_=pk[:, pk_base + b["igo"] : pk_base + b["igo"] + icols],
                )
                nc.sync.dma_start(
                    out=pk_sb[:, icols:],
                    in_=pk[
                        :,
                        pk_base + lp.IGOT + b["cgo"] :
                        pk_base + lp.IGOT + b["cgo"] + nch,
                    ],
                )
                ic = 0
                for q, qq in enumerate(b["quarters"]):
                    if qq["tot"] == 0:
                        continue
                    t0, t1 = tbase_of(q)
                    for s0 in range(0, qq["tot"], 1024):
                        n = min(1024, qq["tot"] - s0)
                        c0 = qq["o"] + s0 // P
                        nc.gpsimd.dma_gather(
                            gbuf[:, c0 : c0 + n // P, :],
                            table[t0:t1, :],
                            pk_sb[:, ic + s0 // 16 : ic + (s0 + n) // 16],
                            n, n, elem,
                            queue_num=qrr[0] % 4,
                        )
                        qrr[0] += 1
                    ic += qq["icols"]
                return pk_sb[:, icols:].bitcast(f16)

            def onehot_and_agg(b, dl_sb, gbuf, ohp, psa, m_dim, tag):
                nch = b["nch"]
                oh = ohp.tile([P, nch, P], f16, tag="oh", name="oh")
                nc.vector.tensor_tensor(
                    out=oh[:],
                    in0=dl_sb.unsqueeze(2).to_broadcast([P, nch, P]),
                    in1=iota_sb[:].unsqueeze(1).to_broadcast([P, nch, P]),
                    op=mybir.AluOpType.is_equal,
                )
                mm = []
                for q, qq in enumerate(b["quarters"]):
                    for (t, ch) in qq["segs"]:
                        mm.append((t, qq["o"] + ch))
                first, last, aggs = {}, {}, {}
                for i, (t, ch) in enumerate(mm):
                    first.setdefault(t, i)
                    last[t] = i
                for t in b["tiles"]:
                    aggs[t] = psa.tile(
                        [m_dim, P], f32, tag=tag, name=f"{tag}_{t}"
                    )
                for i, (t, ch) in enumerate(mm):
                    nc.tensor.matmul(
                        out=aggs[t][:],
                        lhsT=gbuf[:, ch : ch + 1, :],
                        rhs=oh[:, ch : ch + 1, :],
                        start=(i == first[t]),
                        stop=(i == last[t]),
                    )
                return aggs

            # ---------------- phase 1 ----------------
            with (
                tc.tile_pool(name="l1g", bufs=2) as gp,
                tc.tile_pool(name="l1oh", bufs=2) as ohp,
                tc.tile_pool(name="l1sm", bufs=3) as sm,
                tc.tile_pool(name="l1ps", bufs=c.T_B + 1, space="PSUM") as psa,
                tc.tile_pool(name="l1pst", bufs=1, space="PSUM") as pst,
            ):
                state["sm"] = sm
                ag_row = [0]

                def issue_ag(r1):
                    # one collective per completed slice: the AG output
                    # (concat over cores) is contiguous only within a slice
                    r1 = min(r1, SHARD)
                    while ag_row[0] + c.SROW <= r1:
                        r0 = ag_row[0]
                        rs = r0 + c.SROW
                        nc.gpsimd.collective_compute(
                            "AllGather",
                            mybir.AluOpType.bypass,
                            replica_groups=[list(range(c.NCORE))],
                            ins=[z_shard[r0:rs, :].opt()],
                            outs=[
                                z_full[r0 * c.NCORE : rs * c.NCORE, :].opt()
                            ],
                        )
                        ag_row[0] = rs

                for b in L1.batches:
                    if b["nch"] == 0:
                        continue
                    g1 = gp.tile([P, b["nch"], CH], f16, tag="g1", name="g1")
                    dl_sb = gather_batch(
                        L1, b, 0, g1, x16,
                        lambda q: (q * QS, (q + 1) * QS), CH,
                    )
                    aggs = onehot_and_agg(b, dl_sb, g1, ohp, psa, CH, "agg1")
                    for t in b["tiles"]:
                        tw = tile_cols(t)
                        xT_sb = sm.tile([CH, P], f32, tag="xt", name="xt")
                        nc.scalar.dma_start(
                            out=xT_sb[:, :tw], in_=xT[:, t * P : t * P + tw]
                        )
                        sT = sm.tile([CH, P], f16, tag="st", name="st")
                        nc.vector.tensor_add(
                            out=sT[:, :tw], in0=aggs[t][:, :tw],
                            in1=xT_sb[:, :tw],
                        )
                        h1p = pst.tile([H, P], f32, tag="h1", name="h1p")
                        nc.tensor.matmul(
                            out=h1p[:, :tw], lhsT=w1_sb[:], rhs=sT[:, :tw],
                            start=True, stop=True,
                        )
                        h1f = sm.tile([H, P], f16, tag="h1f", name="h1f")
                        nc.scalar.activation(
                            out=h1f[:, :tw], in_=h1p[:, :tw], func=relu,
                            bias=b1_sb[:],
                        )
                        zp = pst.tile([H, P], f32, tag="zp", name="zp")
                        nc.tensor.matmul(
                            out=zp[:, :tw], lhsT=w2_sb[:], rhs=h1f[:, :tw],
                            start=True, stop=True,
                        )
                        nc.vector.tensor_copy(
                            out=zT_sh[:, t * P : t * P + tw], in_=zp[:, :tw]
                        )
                        zf = sm.tile([H, P], f16, tag="zf", name="zf")
                        nc.vector.tensor_copy(out=zf[:, :tw], in_=zp[:, :tw])
                        ztr = pst.tile([P, H], f16, tag="ztr", name="ztr")
                        nc.tensor.transpose(
                            out=ztr[:tw, :], in_=zf[:, :tw], identity=id_sb[:]
                        )
                        zr = sm.tile([P, P], f16, tag="zr", name="zr")
                        nc.vector.memset(zr[:, H:], 0)
                        nc.vector.tensor_copy(out=zr[:tw, :H], in_=ztr[:tw, :])
                        nc.scalar.dma_start(
                            out=z_shard[t * P : t * P + tw, :], in_=zr[:tw, :]
                        )
                    done_rows = (b["tiles"][-1] + 1) * P
                    issue_ag((done_rows // c.SROW) * c.SROW)
                issue_ag(SHARD)

            # ---------------- phase 2 ----------------
            pk2 = L1.IGOT + L1.CGOT
            with (
                tc.tile_pool(name="l2g", bufs=2) as gp,
                tc.tile_pool(name="l2oh", bufs=2) as ohp,
                tc.tile_pool(name="l2sm", bufs=3) as sm,
                tc.tile_pool(name="l2ps", bufs=c.T_B + 1, space="PSUM") as psa,
                tc.tile_pool(name="l2pst", bufs=1, space="PSUM") as pst,
            ):
                state["sm"] = sm
                SR8 = c.SROW * c.NCORE
                for b in L2.batches:
                    if b["nch"] == 0:
                        continue
                    g2 = gp.tile([P, b["nch"], P], f16, tag="g2", name="g2")
                    dl_sb = gather_batch(
                        L2, b, pk2, g2, z_full,
                        lambda s: (s * SR8, (s + 1) * SR8), P,
                    )
                    aggs = onehot_and_agg(b, dl_sb, g2, ohp, psa, P, "agg2")
                    for t in b["tiles"]:
                        tw = tile_cols(t)
                        t2 = sm.tile([H, P], f32, tag="t2", name="t2")
                        nc.vector.tensor_add(
                            out=t2[:, :tw], in0=aggs[t][:H, :tw],
                            in1=zT_sh[:, t * P : t * P + tw],
                        )
                        h2f = sm.tile([H, P], f16, tag="h2f", name="h2f")
                        nc.scalar.activation(
                            out=h2f[:, :tw], in_=t2[:, :tw], func=relu,
                            bias=b2_sb[:],
                        )
                        h3p = pst.tile([16, P], f32, tag="h3", name="h3p")
                        nc.tensor.matmul(
                            out=h3p[:, :tw], lhsT=w3_sb[:], rhs=h2f[:, :tw],
                            start=True, stop=True,
                        )
                        h3f = sm.tile([16, P], f16, tag="h3f", name="h3f")
                        nc.scalar.activation(
                            out=h3f[:, :tw], in_=h3p[:, :tw], func=relu,
                            bias=b3_sb[:],
                        )
                        op_ = pst.tile([1, P], f32, tag="op", name="op_")
                        nc.tensor.matmul(
                            out=op_[:, :tw], lhsT=w4_sb[:], rhs=h3f[:, :tw],
                            start=True, stop=True,
                        )
                        nc.vector.scalar_tensor_tensor(
                            out=outT[:, t * P : t * P + tw],
                            in0=op_[:, :tw],
                            scalar=1.0,
                            in1=b4_sb[:].to_broadcast([1, tw]),
                            op0=mybir.AluOpType.mult,
                            op1=mybir.AluOpType.add,
                        )
            nc.sync.dma_start(out=out[:, :], in_=outT[:])
    nc.compile()
    return nc


def _in_maps(cfg, plan, x, W1, b1, W2, b2, W3, b3, W4, b4):
    c = cfg
    common = dict(
        x16=x.astype(np.float16),
        iota=np.broadcast_to(np.arange(P, dtype=np.float16), (P, P)).copy(),
        ident=np.eye(c.H, dtype=np.float16),
        w1=W1.astype(np.float16),
        w2=W2.astype(np.float16),
        w3=W3.astype(np.float16),
        w4=W4.astype(np.float16),
        b1=b1.reshape(-1, 1).astype(np.float32),
        b2=b2.reshape(-1, 1).astype(np.float32),
        b3=b3.reshape(-1, 1).astype(np.float32),
        b4v=b4.reshape(1, 1).astype(np.float32),
    )
    in_maps = []
    for ci in range(c.NCORE):
        pk_a = plan.core_arrays(ci)
        xT_a = np.ascontiguousarray(
            x[ci * c.SHARD : (ci + 1) * c.SHARD].T.astype(np.float32)
        )
        in_maps.append(dict(common, pk=pk_a, xT=xT_a))
    return in_maps


def _run(cfg, plan, nc, x, W1, b1, W2, b2, W3, b3, W4, b4, **kw):
    from concourse.bass_utils import run_bass_kernel_spmd

    c = cfg
    in_maps = _in_maps(cfg, plan, x, W1, b1, W2, b2, W3, b3, W4, b4)
    res = run_bass_kernel_spmd(nc, in_maps, core_ids=list(range(c.NCORE)), **kw)
    outs = [res.results[ci]["out"].reshape(-1) for ci in range(c.NCORE)]
    return np.concatenate(outs).reshape(-1, 1).astype(np.float32), res


def kernel(x, edge_index, W1, b1, W2, b2, W3, b3, W4, b4):
    cfg = FULL
    x = np.asarray(x, dtype=np.float32)
    src = np.asarray(edge_index[0], dtype=np.int64)
    dst = np.asarray(edge_index[1], dtype=np.int64)
    plan = Plan(cfg, src, dst)
    nc = _build(plan)
    out, _ = _run(
        cfg, plan, nc, x,
        np.asarray(W1), np.asarray(b1), np.asarray(W2), np.asarray(b2),
        np.asarray(W3), np.asarray(b3), np.asarray(W4), np.asarray(b4),
    )
    return out



# revision 3
# speedup vs baseline: 1.0309x; 1.0309x over previous
"""GIN message-passing (2 GINConv layers + 2 linear) on 8 TRN2 NeuronCores.

Strategy (self-contained; shapes hardcoded for the 100k-node / 1.6M-edge
problem):
  - Shard dst nodes across 8 cores (12500 each); dst tiles of 128 nodes.
  - Per-edge source rows fetched with gpsimd.dma_gather (int16 indices,
    256B rows, <=1024 idx/call, calls round-robin over 4 SWDGE queues).
    The gather is descriptor-count-bound (~3.1ns/desc), so slot count is
    minimized: per-(tile,group) budgets are max-over-cores WITHOUT x128
    rounding; x128 padding only per (tile-batch, group) work unit. Chunks
    may span two adjacent tiles: second tile's slots get dl+128 and a
    second one-hot built against iota+128.
  - Aggregation via one-hot matmuls: agg[ch, dst] += gbuf[slots, ch].T @
    onehot[slots, dst] accumulated in PSUM per tile.
  - Layer 1 (batch-major): gathers x quarters (node-id groups of 25000);
    finalize computes z = relu((x + A@x)@W1 + b1), u = z@W2; u rows (f16,
    padded to 128 ch) written to HBM; sliced AllGather (overlapped with
    phase 1) shares u: z_full layout [slice][core][row].
  - Layer 2 (group-major, so gathers of slice g start as soon as its
    AllGather lands): gathers u rows, accumulates into an SBUF aggregate;
    on the last group pass finalizes h2 = relu(u_dst + A@u + b2),
    h3 = relu(h2@W3+b3), out = h3@W4+b4, written per tile.
"""

import numpy as np

P = 128


class Cfg:
    def __init__(self, n_nodes, n_edges, in_ch, hid, n_cores, t_b, n_slices=4):
        self.N = n_nodes
        self.E = n_edges
        self.CH = in_ch
        self.H = hid
        self.NCORE = n_cores
        self.SHARD = n_nodes // n_cores
        self.NT = -(-self.SHARD // P)
        self.NQ = 4  # x16 table groups (quarters)
        self.QS = n_nodes // 4
        assert self.QS <= 32768
        self.NS = n_slices  # z table groups (AG slices)
        assert self.SHARD % n_slices == 0
        self.SROW = self.SHARD // n_slices  # shard rows per slice
        assert self.SROW * n_cores <= 32768
        self.T_B = t_b
        self.NB = -(-self.NT // t_b)


FULL = Cfg(100000, 1600000, 128, 64, 8, 4)


def _r128(a):
    return ((a + 127) // 128) * 128


class LayerPlan:
    """Gather schedule for one layer as a list of work units (tile-batch x
    group). Slot budgets per (tile, group) are max over cores (no rounding);
    each unit's total is padded to x128. group_major orders units (g, b)
    instead of (b, g)."""

    def __init__(self, cfg, n_groups, per_core_edges, group_major):
        c = cfg
        self.cfg = c
        self.NG = n_groups
        self.group_major = group_major
        counts = np.zeros((c.NCORE, c.NT, n_groups), dtype=np.int64)
        self.groups = []
        for ci in range(c.NCORE):
            t, grp, lidx, dl = per_core_edges[ci]
            key = t * n_groups + grp
            order = np.argsort(key, kind="stable")
            key = key[order]
            lidx = lidx[order].astype(np.int16)
            dl = dl[order].astype(np.int16)
            cnt = np.bincount(key, minlength=c.NT * n_groups).reshape(
                c.NT, n_groups
            )
            counts[ci] = cnt
            bounds = np.concatenate([[0], np.cumsum(cnt.reshape(-1))])
            g = {}
            for tt in range(c.NT):
                for qq in range(n_groups):
                    k = tt * n_groups + qq
                    lo, hi = bounds[k], bounds[k + 1]
                    if hi > lo:
                        g[(tt, qq)] = (lidx[lo:hi], dl[lo:hi])
            self.groups.append(g)

        self.B = counts.max(axis=0)  # [NT, NG]
        assert (self.B >= P).all(), "chunk would span 3 tiles"

        batches = [
            list(range(b * c.T_B, min((b + 1) * c.T_B, c.NT)))
            for b in range(c.NB)
        ]
        if group_major:
            order = [(b, q) for q in range(n_groups) for b in range(c.NB)]
        else:
            order = [(b, q) for b in range(c.NB) for q in range(n_groups)]
        self.units = []
        igo = 0
        cgo = 0
        for (b, q) in order:
            tiles = batches[b]
            spans = {}
            s0 = 0
            for t in tiles:
                spans[t] = (s0, s0 + int(self.B[t, q]))
                s0 += int(self.B[t, q])
            tot = _r128(s0)
            nch = tot // P
            # owner tile of each chunk's first slot
            tbase = np.zeros(nch, dtype=np.int64)
            for t in tiles:
                lo, hi = spans[t]
                for ch in range(lo // P, (hi - 1) // P + 1):
                    if ch * P >= lo:
                        tbase[ch] = t
            for ch in range(nch):
                if ch * P >= s0:
                    tbase[ch] = tiles[-1]
            pieces = []  # (ch, t, var)
            bnd = []
            for t in tiles:
                lo, hi = spans[t]
                for ch in range(lo // P, (hi - 1) // P + 1):
                    var = t - tbase[ch]
                    assert var in (0, 1)
                    if var == 1:
                        bnd.append(ch)
                    pieces.append((ch, t, var))
            self.units.append(
                dict(b=b, grp=q, tiles=tiles, spans=spans, tot=tot, nch=nch,
                     tbase=tbase, pieces=pieces, bnd=bnd, igo=igo, cgo=cgo)
            )
            igo += tot // 16
            cgo += nch
        self.IGOT = igo
        self.CGOT = cgo

    def core_arrays(self, ci):
        c = self.cfg
        idx16 = np.zeros((P, self.IGOT), dtype=np.int16)
        dstloc = np.full((P, self.CGOT), -1.0, dtype=np.float16)
        g = self.groups[ci]
        for u in self.units:
            tot, nch, q = u["tot"], u["nch"], u["grp"]
            sl_full = np.zeros(tot, dtype=np.int16)
            dl_full = np.full(tot, -1.0, dtype=np.float16)
            for t in u["tiles"]:
                lo, hi = u["spans"][t]
                if (t, q) not in g:
                    continue
                sl, dl = g[(t, q)]
                n = len(sl)
                pos = lo + np.arange(n)
                sl_full[pos] = sl
                dl_full[pos] = (
                    (t - u["tbase"][pos // P]) * P + dl
                ).astype(np.float16)
            blk = sl_full.reshape(tot // 16, 16).T
            icol = u["igo"]
            idx16[:, icol : icol + tot // 16] = np.tile(blk, (8, 1))
            dstloc[:, u["cgo"] : u["cgo"] + nch] = dl_full.reshape(nch, P).T
        return idx16, dstloc


class Plan:
    def __init__(self, cfg, src, dst):
        c = cfg
        self.cfg = c
        core = dst // c.SHARD
        e1, e2 = [], []
        for ci in range(c.NCORE):
            m = core == ci
            s = src[m]
            d = dst[m] - ci * c.SHARD
            t = d // P
            dl = d % P
            # layer 1: x16 table grouped by quarters of node id
            e1.append((t, s // c.QS, s % c.QS, dl))
            # layer 2: z_full is [slice][core][row]; slice = shard-row/SROW
            sc = s // c.SHARD
            sr = s % c.SHARD
            sl = sr // c.SROW
            lidx = sc * c.SROW + (sr - sl * c.SROW)
            e2.append((t, sl, lidx, dl))
        self.L1 = LayerPlan(cfg, c.NQ, e1, group_major=False)
        self.L2 = LayerPlan(cfg, c.NS, e2, group_major=True)

    def core_arrays(self, ci):
        i1, d1 = self.L1.core_arrays(ci)
        i2, d2 = self.L2.core_arrays(ci)
        return np.concatenate(
            [i1, d1.view(np.int16), i2, d2.view(np.int16)], axis=1
        )


def _build(plan):
    import concourse.tile as tile
    from concourse import bacc, mybir

    c = plan.cfg
    f16, f32, i16 = mybir.dt.float16, mybir.dt.float32, mybir.dt.int16
    CH, H, QS, SHARD = c.CH, c.H, c.QS, c.SHARD
    L1, L2 = plan.L1, plan.L2
    PKW = L1.IGOT + L1.CGOT + L2.IGOT + L2.CGOT

    nc = bacc.Bacc(
        "TRN2", target_bir_lowering=False, debug=False, num_devices=c.NCORE,
        num_swdge_queues=4,
    )
    x16 = nc.dram_tensor("x16", [c.N, CH], f16, kind="ExternalInput")
    xT = nc.dram_tensor("xT", [CH, SHARD], f32, kind="ExternalInput")
    pk = nc.dram_tensor("pk", [P, PKW], i16, kind="ExternalInput")
    iota = nc.dram_tensor("iota", [P, P], f16, kind="ExternalInput")
    iota2 = nc.dram_tensor("iota2", [P, P], f16, kind="ExternalInput")
    ident = nc.dram_tensor("ident", [H, H], f16, kind="ExternalInput")
    w1 = nc.dram_tensor("w1", [CH, H], f16, kind="ExternalInput")
    w2 = nc.dram_tensor("w2", [H, H], f16, kind="ExternalInput")
    w3 = nc.dram_tensor("w3", [H, 16], f16, kind="ExternalInput")
    w4 = nc.dram_tensor("w4", [16, 1], f16, kind="ExternalInput")
    b1 = nc.dram_tensor("b1", [H, 1], f32, kind="ExternalInput")
    b2 = nc.dram_tensor("b2", [H, 1], f32, kind="ExternalInput")
    b3 = nc.dram_tensor("b3", [16, 1], f32, kind="ExternalInput")
    b4v = nc.dram_tensor("b4v", [1, 1], f32, kind="ExternalInput")
    out = nc.dram_tensor("out", [1, SHARD], f32, kind="ExternalOutput")

    with tile.TileContext(nc) as tc:
        with (
            tc.tile_pool(name="const", bufs=1) as cp,
            tc.tile_pool(name="persist", bufs=1) as pp,
            tc.tile_pool(name="dram", bufs=1, space="DRAM") as dp,
        ):
            def load_const(name, t, shape, dt):
                sb = cp.tile(shape, dt, name=name + "_sb")
                nc.sync.dma_start(out=sb[:], in_=t[:, :])
                return sb

            iota_sb = load_const("iota", iota, [P, P], f16)
            iota2_sb = load_const("iota2", iota2, [P, P], f16)
            id_sb = load_const("id", ident, [H, H], f16)
            w1_sb = load_const("w1", w1, [CH, H], f16)
            w2_sb = load_const("w2", w2, [H, H], f16)
            w3_sb = load_const("w3", w3, [H, 16], f16)
            w4_sb = load_const("w4", w4, [16, 1], f16)
            b1_sb = load_const("b1", b1, [H, 1], f32)
            b2_sb = load_const("b2", b2, [H, 1], f32)
            b3_sb = load_const("b3", b3, [16, 1], f32)
            b4_sb = load_const("b4", b4v, [1, 1], f32)

            zT_sh = pp.tile([H, SHARD], f16)
            aggS2 = pp.tile([H, SHARD], f32)
            z_shard = dp.tile([SHARD, P], f16)
            z_full = dp.tile([c.N, P], f16)  # [slice][core][row][ch]

            relu = mybir.ActivationFunctionType.Relu
            qrr = [0]  # round-robin SWDGE queue
            state = {}

            def tile_cols(t):
                return min(P, SHARD - t * P)

            def gather_unit(lp, u, pk_base, gbuf, table, t0, t1, elem):
                sm = state["sm"]
                icols = u["tot"] // 16
                nch = u["nch"]
                pk_sb = sm.tile([P, icols + nch], i16, tag="pk")
                nc.sync.dma_start(
                    out=pk_sb[:, :icols],
                    in_=pk[:, pk_base + u["igo"] : pk_base + u["igo"] + icols],
                )
                nc.sync.dma_start(
                    out=pk_sb[:, icols:],
                    in_=pk[
                        :,
                        pk_base + lp.IGOT + u["cgo"] :
                        pk_base + lp.IGOT + u["cgo"] + nch,
                    ],
                )
                for s0 in range(0, u["tot"], 1024):
                    n = min(1024, u["tot"] - s0)
                    nc.gpsimd.dma_gather(
                        gbuf[:, s0 // P : (s0 + n) // P, :],
                        table[t0:t1, :],
                        pk_sb[:, s0 // 16 : (s0 + n) // 16],
                        n, n, elem,
                        queue_num=qrr[0] % 4,
                    )
                    qrr[0] += 1
                return pk_sb[:, icols:].bitcast(f16)

            def build_oh(u, dl_sb, ohp, tag):
                nch = u["nch"]
                oh = ohp.tile([P, nch, P], f16, tag=tag, name="oh" + tag)
                nc.vector.tensor_tensor(
                    out=oh[:],
                    in0=dl_sb.unsqueeze(2).to_broadcast([P, nch, P]),
                    in1=iota_sb[:].unsqueeze(1).to_broadcast([P, nch, P]),
                    op=mybir.AluOpType.is_equal,
                )
                ohn = None
                if u["bnd"]:
                    nb = len(u["bnd"])
                    ohn = ohp.tile([P, nb, P], f16, tag=tag + "n",
                                   name="ohn" + tag)
                    for j, ch in enumerate(u["bnd"]):
                        nc.vector.tensor_tensor(
                            out=ohn[:, j : j + 1, :],
                            in0=dl_sb[:, ch : ch + 1].unsqueeze(2)
                            .to_broadcast([P, 1, P]),
                            in1=iota2_sb[:].unsqueeze(1)
                            .to_broadcast([P, 1, P]),
                            op=mybir.AluOpType.is_equal,
                        )
                return oh, ohn

            # ---------------- phase 1 (batch-major) ----------------
            with (
                tc.tile_pool(name="l1g", bufs=2) as gp,
                tc.tile_pool(name="l1oh", bufs=2) as ohp,
                tc.tile_pool(name="l1sm", bufs=3) as sm,
                tc.tile_pool(name="l1ps", bufs=c.T_B + 1, space="PSUM") as psa,
                tc.tile_pool(name="l1pst", bufs=1, space="PSUM") as pst,
            ):
                state["sm"] = sm
                ag_row = [0]

                def issue_ag(r1):
                    r1 = min(r1, SHARD)
                    while ag_row[0] + c.SROW <= r1:
                        r0 = ag_row[0]
                        rs = r0 + c.SROW
                        nc.gpsimd.collective_compute(
                            "AllGather",
                            mybir.AluOpType.bypass,
                            replica_groups=[list(range(c.NCORE))],
                            ins=[z_shard[r0:rs, :].opt()],
                            outs=[
                                z_full[r0 * c.NCORE : rs * c.NCORE, :].opt()
                            ],
                        )
                        ag_row[0] = rs

                for b in range(c.NB):
                    units = [L1.units[b * c.NQ + q] for q in range(c.NQ)]
                    assert all(u["b"] == b for u in units)
                    gbufs, ohs, ohns = [], [], []
                    for q, u in enumerate(units):
                        gb = gp.tile([P, u["nch"], CH], f16, tag=f"g{q}",
                                     name=f"g1_{q}")
                        dl_sb = gather_unit(
                            L1, u, 0, gb, x16, q * QS, (q + 1) * QS, CH)
                        oh, ohn = build_oh(u, dl_sb, ohp, f"o{q}")
                        gbufs.append(gb)
                        ohs.append(oh)
                        ohns.append(ohn)
                    # piece list across the batch's 4 units
                    mm = []
                    for q, u in enumerate(units):
                        bnd_ix = {ch: j for j, ch in enumerate(u["bnd"])}
                        for (ch, t, var) in u["pieces"]:
                            mm.append((q, ch, t, var, bnd_ix.get(ch)))
                    first, last, aggs = {}, {}, {}
                    for i, (q, ch, t, var, j) in enumerate(mm):
                        first.setdefault(t, i)
                        last[t] = i
                    for t in units[0]["tiles"]:
                        aggs[t] = psa.tile([CH, P], f32, tag="agg1",
                                           name=f"agg1_{t}")
                    for i, (q, ch, t, var, j) in enumerate(mm):
                        rhs = (ohs[q][:, ch : ch + 1, :] if var == 0
                               else ohns[q][:, j : j + 1, :])
                        nc.tensor.matmul(
                            out=aggs[t][:],
                            lhsT=gbufs[q][:, ch : ch + 1, :],
                            rhs=rhs,
                            start=(i == first[t]),
                            stop=(i == last[t]),
                        )
                    for t in units[0]["tiles"]:
                        tw = tile_cols(t)
                        xT_sb = sm.tile([CH, P], f32, tag="xt", name="xt")
                        nc.scalar.dma_start(
                            out=xT_sb[:, :tw], in_=xT[:, t * P : t * P + tw]
                        )
                        sT = sm.tile([CH, P], f16, tag="st", name="st")
                        nc.vector.tensor_add(
                            out=sT[:, :tw], in0=aggs[t][:, :tw],
                            in1=xT_sb[:, :tw],
                        )
                        h1p = pst.tile([H, P], f32, tag="h1", name="h1p")
                        nc.tensor.matmul(
                            out=h1p[:, :tw], lhsT=w1_sb[:], rhs=sT[:, :tw],
                            start=True, stop=True,
                        )
                        h1f = sm.tile([H, P], f16, tag="h1f", name="h1f")
                        nc.scalar.activation(
                            out=h1f[:, :tw], in_=h1p[:, :tw], func=relu,
                            bias=b1_sb[:],
                        )
                        zp = pst.tile([H, P], f32, tag="zp", name="zp")
                        nc.tensor.matmul(
                            out=zp[:, :tw], lhsT=w2_sb[:], rhs=h1f[:, :tw],
                            start=True, stop=True,
                        )
                        nc.vector.tensor_copy(
                            out=zT_sh[:, t * P : t * P + tw], in_=zp[:, :tw]
                        )
                        zf = sm.tile([H, P], f16, tag="zf", name="zf")
                        nc.vector.tensor_copy(out=zf[:, :tw], in_=zp[:, :tw])
                        ztr = pst.tile([P, H], f16, tag="ztr", name="ztr")
                        nc.tensor.transpose(
                            out=ztr[:tw, :], in_=zf[:, :tw], identity=id_sb[:]
                        )
                        zr = sm.tile([P, P], f16, tag="zr", name="zr")
                        nc.vector.memset(zr[:, H:], 0)
                        nc.vector.tensor_copy(out=zr[:tw, :H], in_=ztr[:tw, :])
                        nc.scalar.dma_start(
                            out=z_shard[t * P : t * P + tw, :], in_=zr[:tw, :]
                        )
                    done_rows = (units[0]["tiles"][-1] + 1) * P
                    issue_ag((done_rows // c.SROW) * c.SROW)
                issue_ag(SHARD)

            # ---------------- phase 2 (group-major) ----------------
            pk2 = L1.IGOT + L1.CGOT
            with (
                tc.tile_pool(name="l2g", bufs=3) as gp,
                tc.tile_pool(name="l2oh", bufs=3) as ohp,
                tc.tile_pool(name="l2sm", bufs=4) as sm,
                tc.tile_pool(name="l2ps", bufs=c.T_B + 1, space="PSUM") as psa,
                tc.tile_pool(name="l2pst", bufs=1, space="PSUM") as pst,
            ):
                state["sm"] = sm
                SR8 = c.SROW * c.NCORE
                for ui, u in enumerate(L2.units):
                    g = u["grp"]
                    assert ui // c.NB == g
                    g2 = gp.tile([P, u["nch"], P], f16, tag="g2", name="g2")
                    dl_sb = gather_unit(
                        L2, u, pk2, g2, z_full, g * SR8, (g + 1) * SR8, P)
                    oh, ohn = build_oh(u, dl_sb, ohp, "o2")
                    bnd_ix = {ch: j for j, ch in enumerate(u["bnd"])}
                    first, last, aggs = {}, {}, {}
                    for i, (ch, t, var) in enumerate(u["pieces"]):
                        first.setdefault(t, i)
                        last[t] = i
                    for t in u["tiles"]:
                        aggs[t] = psa.tile([H, P], f32, tag="agg2",
                                           name=f"agg2_{t}")
                    for i, (ch, t, var) in enumerate(u["pieces"]):
                        rhs = (oh[:, ch : ch + 1, :] if var == 0
                               else ohn[:, bnd_ix[ch] : bnd_ix[ch] + 1, :])
                        nc.tensor.matmul(
                            out=aggs[t][:],
                            lhsT=g2[:, ch : ch + 1, :H],
                            rhs=rhs,
                            start=(i == first[t]),
                            stop=(i == last[t]),
                        )
                    for t in u["tiles"]:
                        tw = tile_cols(t)
                        cols = slice(t * P, t * P + tw)
                        if g == 0:
                            nc.vector.tensor_copy(
                                out=aggS2[:, cols], in_=aggs[t][:, :tw]
                            )
                        else:
                            nc.vector.tensor_add(
                                out=aggS2[:, cols], in0=aggS2[:, cols],
                                in1=aggs[t][:, :tw],
                            )
                        if g == c.NS - 1:
                            t2 = sm.tile([H, P], f32, tag="t2", name="t2")
                            nc.vector.tensor_add(
                                out=t2[:, :tw], in0=aggS2[:, cols],
                                in1=zT_sh[:, cols],
                            )
                            h2f = sm.tile([H, P], f16, tag="h2f", name="h2f")
                            nc.scalar.activation(
                                out=h2f[:, :tw], in_=t2[:, :tw], func=relu,
                                bias=b2_sb[:],
                            )
                            h3p = pst.tile([16, P], f32, tag="h3", name="h3p")
                            nc.tensor.matmul(
                                out=h3p[:, :tw], lhsT=w3_sb[:],
                                rhs=h2f[:, :tw], start=True, stop=True,
                            )
                            h3f = sm.tile([16, P], f16, tag="h3f", name="h3f")
                            nc.scalar.activation(
                                out=h3f[:, :tw], in_=h3p[:, :tw], func=relu,
                                bias=b3_sb[:],
                            )
                            op_ = pst.tile([1, P], f32, tag="op", name="op_")
                            nc.tensor.matmul(
                                out=op_[:, :tw], lhsT=w4_sb[:],
                                rhs=h3f[:, :tw], start=True, stop=True,
                            )
                            ot = sm.tile([1, P], f32, tag="ot", name="ot")
                            nc.vector.scalar_tensor_tensor(
                                out=ot[:, :tw],
                                in0=op_[:, :tw],
                                scalar=1.0,
                                in1=b4_sb[:].to_broadcast([1, tw]),
                                op0=mybir.AluOpType.mult,
                                op1=mybir.AluOpType.add,
                            )
                            nc.scalar.dma_start(
                                out=out[:, t * P : t * P + tw],
                                in_=ot[:, :tw],
                            )
    nc.compile()
    return nc


def _in_maps(cfg, plan, x, W1, b1, W2, b2, W3, b3, W4, b4):
    c = cfg
    iota_a = np.broadcast_to(
        np.arange(P, dtype=np.float16), (P, P)).copy()
    common = dict(
        x16=x.astype(np.float16),
        iota=iota_a,
        iota2=iota_a + np.float16(128.0),
        ident=np.eye(c.H, dtype=np.float16),
        w1=W1.astype(np.float16),
        w2=W2.astype(np.float16),
        w3=W3.astype(np.float16),
        w4=W4.astype(np.float16),
        b1=b1.reshape(-1, 1).astype(np.float32),
        b2=b2.reshape(-1, 1).astype(np.float32),
        b3=b3.reshape(-1, 1).astype(np.float32),
        b4v=b4.reshape(1, 1).astype(np.float32),
    )
    in_maps = []
    for ci in range(c.NCORE):
        pk_a = plan.core_arrays(ci)
        xT_a = np.ascontiguousarray(
            x[ci * c.SHARD : (ci + 1) * c.SHARD].T.astype(np.float32)
        )
        in_maps.append(dict(common, pk=pk_a, xT=xT_a))
    return in_maps


def _run(cfg, plan, nc, x, W1, b1, W2, b2, W3, b3, W4, b4, **kw):
    from concourse.bass_utils import run_bass_kernel_spmd

    c = cfg
    in_maps = _in_maps(cfg, plan, x, W1, b1, W2, b2, W3, b3, W4, b4)
    res = run_bass_kernel_spmd(nc, in_maps, core_ids=list(range(c.NCORE)), **kw)
    outs = [res.results[ci]["out"].reshape(-1) for ci in range(c.NCORE)]
    return np.concatenate(outs).reshape(-1, 1).astype(np.float32), res


def kernel(x, edge_index, W1, b1, W2, b2, W3, b3, W4, b4):
    cfg = FULL
    x = np.asarray(x, dtype=np.float32)
    src = np.asarray(edge_index[0], dtype=np.int64)
    dst = np.asarray(edge_index[1], dtype=np.int64)
    plan = Plan(cfg, src, dst)
    nc = _build(plan)
    out, _ = _run(
        cfg, plan, nc, x,
        np.asarray(W1), np.asarray(b1), np.asarray(W2), np.asarray(b2),
        np.asarray(W3), np.asarray(b3), np.asarray(W4), np.asarray(b4),
    )
    return out


# revision 5
# speedup vs baseline: 1.2422x; 1.2049x over previous
"""GIN message-passing (2 GINConv layers + 2 linear) on 8 TRN2 NeuronCores.

Strategy (self-contained; shapes hardcoded for the 100k-node / 1.6M-edge
problem):
  - Shard dst nodes across 8 cores (12500 each); dst tiles of 128 nodes.
  - Per-edge source rows fetched with gpsimd.dma_gather (int16 indices,
    256B rows, <=1024 idx/call, calls round-robin over 4 SWDGE queues).
    The gather is descriptor-count-bound (~3.1ns/desc), so slot count is
    minimized: per-(tile,group) budgets are max-over-cores WITHOUT x128
    rounding; x128 padding only per (tile-batch, group) work unit. Chunks
    may span two adjacent tiles: second tile's slots get dl+128 and a
    second one-hot built against iota+128.
  - Aggregation via one-hot matmuls: agg[ch, dst] += gbuf[slots, ch].T @
    onehot[slots, dst] accumulated in PSUM per tile.
  - Layer 1 (batch-major): gathers x quarters (node-id groups of 25000);
    finalize computes z = relu((x + A@x)@W1 + b1), u = z@W2; u rows (f16,
    padded to 128 ch) written to HBM; sliced AllGather (overlapped with
    phase 1) shares u: z_full layout [slice][core][row].
  - Layer 2 (group-major, so gathers of slice g start as soon as its
    AllGather lands): gathers u rows, accumulates into an SBUF aggregate;
    on the last group pass finalizes h2 = relu(u_dst + A@u + b2),
    h3 = relu(h2@W3+b3), out = h3@W4+b4, written per tile.
"""

import numpy as np

P = 128


class Cfg:
    def __init__(self, n_nodes, n_edges, in_ch, hid, n_cores, t_b, n_slices=4):
        self.N = n_nodes
        self.E = n_edges
        self.CH = in_ch
        self.H = hid
        self.NCORE = n_cores
        self.SHARD = n_nodes // n_cores
        self.NT = -(-self.SHARD // P)
        self.NQ = 4  # x16 table groups (quarters)
        self.QS = n_nodes // 4
        assert self.QS <= 32768
        self.NS = n_slices  # z table groups (AG slices)
        assert self.SHARD % n_slices == 0
        self.SROW = self.SHARD // n_slices  # shard rows per slice
        assert self.SROW * n_cores <= 32768
        self.T_B = t_b
        self.NB = -(-self.NT // t_b)


FULL = Cfg(100000, 1600000, 128, 64, 8, 4)


def _r128(a):
    return ((a + 127) // 128) * 128


class LayerPlan:
    """Gather schedule for one layer as a list of work units (tile-batch x
    group). Slot budgets per (tile, group) are max over cores (no rounding);
    each unit's total is padded to x128. group_major orders units (g, b)
    instead of (b, g)."""

    def __init__(self, cfg, n_groups, per_core_edges, group_major):
        c = cfg
        self.cfg = c
        self.NG = n_groups
        self.group_major = group_major
        counts = np.zeros((c.NCORE, c.NT, n_groups), dtype=np.int64)
        self.groups = []
        for ci in range(c.NCORE):
            t, grp, lidx, dl = per_core_edges[ci]
            key = t * n_groups + grp
            order = np.argsort(key, kind="stable")
            key = key[order]
            lidx = lidx[order].astype(np.int16)
            dl = dl[order].astype(np.int16)
            cnt = np.bincount(key, minlength=c.NT * n_groups).reshape(
                c.NT, n_groups
            )
            counts[ci] = cnt
            bounds = np.concatenate([[0], np.cumsum(cnt.reshape(-1))])
            g = {}
            for tt in range(c.NT):
                for qq in range(n_groups):
                    k = tt * n_groups + qq
                    lo, hi = bounds[k], bounds[k + 1]
                    if hi > lo:
                        g[(tt, qq)] = (lidx[lo:hi], dl[lo:hi])
            self.groups.append(g)

        self.B = counts.max(axis=0)  # [NT, NG]
        assert (self.B >= P).all(), "chunk would span 3 tiles"

        batches = [
            list(range(b * c.T_B, min((b + 1) * c.T_B, c.NT)))
            for b in range(c.NB)
        ]
        if group_major:
            order = [(b, q) for q in range(n_groups) for b in range(c.NB)]
        else:
            order = [(b, q) for b in range(c.NB) for q in range(n_groups)]
        self.units = []
        igo = 0
        cgo = 0
        bgo = 0
        for (b, q) in order:
            tiles = batches[b]
            spans = {}
            s0 = 0
            for t in tiles:
                spans[t] = (s0, s0 + int(self.B[t, q]))
                s0 += int(self.B[t, q])
            tot = _r128(s0)
            nch = tot // P
            # owner tile of each chunk's first slot
            tbase = np.zeros(nch, dtype=np.int64)
            for t in tiles:
                lo, hi = spans[t]
                for ch in range(lo // P, (hi - 1) // P + 1):
                    if ch * P >= lo:
                        tbase[ch] = t
            for ch in range(nch):
                if ch * P >= s0:
                    tbase[ch] = tiles[-1]
            pieces = []  # (ch, t, var)
            bnd = []
            for t in tiles:
                lo, hi = spans[t]
                for ch in range(lo // P, (hi - 1) // P + 1):
                    var = t - tbase[ch]
                    assert var in (0, 1)
                    if var == 1:
                        bnd.append(ch)
                    pieces.append((ch, t, var))
            self.units.append(
                dict(b=b, grp=q, tiles=tiles, spans=spans, tot=tot, nch=nch,
                     tbase=tbase, pieces=pieces, bnd=bnd, igo=igo, cgo=cgo,
                     bgo=bgo)
            )
            igo += tot // 16
            cgo += nch
            bgo += len(bnd)
        self.IGOT = igo
        self.CGOT = cgo
        self.BGOT = bgo

    def core_arrays(self, ci):
        c = self.cfg
        idx16 = np.zeros((P, self.IGOT), dtype=np.int16)
        dstloc = np.full((P, self.CGOT + self.BGOT), -1.0, dtype=np.float16)
        g = self.groups[ci]
        for u in self.units:
            tot, nch, q = u["tot"], u["nch"], u["grp"]
            sl_full = np.zeros(tot, dtype=np.int16)
            dl_full = np.full(tot, -1.0, dtype=np.float16)
            for t in u["tiles"]:
                lo, hi = u["spans"][t]
                if (t, q) not in g:
                    continue
                sl, dl = g[(t, q)]
                n = len(sl)
                pos = lo + np.arange(n)
                sl_full[pos] = sl
                dl_full[pos] = (
                    (t - u["tbase"][pos // P]) * P + dl
                ).astype(np.float16)
            blk = sl_full.reshape(tot // 16, 16).T
            icol = u["igo"]
            idx16[:, icol : icol + tot // 16] = np.tile(blk, (8, 1))
            dlc = dl_full.reshape(nch, P).T
            dstloc[:, u["cgo"] : u["cgo"] + nch] = dlc
            for j, ch in enumerate(u["bnd"]):
                dstloc[:, self.CGOT + u["bgo"] + j] = dlc[:, ch]
        return idx16, dstloc


class Plan:
    def __init__(self, cfg, src, dst):
        c = cfg
        self.cfg = c
        core = dst // c.SHARD
        e1, e2 = [], []
        for ci in range(c.NCORE):
            m = core == ci
            s = src[m]
            d = dst[m] - ci * c.SHARD
            t = d // P
            dl = d % P
            # layer 1: x16 table grouped by quarters of node id
            e1.append((t, s // c.QS, s % c.QS, dl))
            # layer 2: z_full is [slice][core][row]; slice = shard-row/SROW
            sc = s // c.SHARD
            sr = s % c.SHARD
            sl = sr // c.SROW
            lidx = sc * c.SROW + (sr - sl * c.SROW)
            e2.append((t, sl, lidx, dl))
        self.L1 = LayerPlan(cfg, c.NQ, e1, group_major=False)
        self.L2 = LayerPlan(cfg, c.NS, e2, group_major=True)

    def core_arrays(self, ci):
        i1, d1 = self.L1.core_arrays(ci)
        i2, d2 = self.L2.core_arrays(ci)
        return np.concatenate(
            [i1, d1.view(np.int16), i2, d2.view(np.int16)], axis=1
        )


def _build(plan):
    import concourse.tile as tile
    from concourse import bacc, mybir

    c = plan.cfg
    f16, f32, i16 = mybir.dt.float16, mybir.dt.float32, mybir.dt.int16
    CH, H, QS, SHARD = c.CH, c.H, c.QS, c.SHARD
    L1, L2 = plan.L1, plan.L2
    PKW = (L1.IGOT + L1.CGOT + L1.BGOT
           + L2.IGOT + L2.CGOT + L2.BGOT)

    nc = bacc.Bacc(
        "TRN2", target_bir_lowering=False, debug=False, num_devices=c.NCORE,
        num_swdge_queues=4,
    )
    x16 = nc.dram_tensor("x16", [c.N, CH], f16, kind="ExternalInput")
    xT = nc.dram_tensor("xT", [CH, SHARD], f32, kind="ExternalInput")
    pk = nc.dram_tensor("pk", [P, PKW], i16, kind="ExternalInput")
    iota = nc.dram_tensor("iota", [P, P], f16, kind="ExternalInput")
    iota2 = nc.dram_tensor("iota2", [P, P], f16, kind="ExternalInput")
    ident = nc.dram_tensor("ident", [H, H], f16, kind="ExternalInput")
    w1 = nc.dram_tensor("w1", [CH, H], f16, kind="ExternalInput")
    w2 = nc.dram_tensor("w2", [H, H], f16, kind="ExternalInput")
    w3 = nc.dram_tensor("w3", [H, 16], f16, kind="ExternalInput")
    w4 = nc.dram_tensor("w4", [16, 1], f16, kind="ExternalInput")
    b1 = nc.dram_tensor("b1", [H, 1], f32, kind="ExternalInput")
    b2 = nc.dram_tensor("b2", [H, 1], f32, kind="ExternalInput")
    b3 = nc.dram_tensor("b3", [16, 1], f32, kind="ExternalInput")
    b4v = nc.dram_tensor("b4v", [1, 1], f32, kind="ExternalInput")
    out = nc.dram_tensor("out", [1, SHARD], f32, kind="ExternalOutput")

    with tile.TileContext(nc) as tc:
        with (
            tc.tile_pool(name="const", bufs=1) as cp,
            tc.tile_pool(name="persist", bufs=1) as pp,
            tc.tile_pool(name="dram", bufs=1, space="DRAM") as dp,
        ):
            def load_const(name, t, shape, dt):
                sb = cp.tile(shape, dt, name=name + "_sb")
                nc.sync.dma_start(out=sb[:], in_=t[:, :])
                return sb

            iota_sb = load_const("iota", iota, [P, P], f16)
            iota2_sb = load_const("iota2", iota2, [P, P], f16)
            id_sb = load_const("id", ident, [H, H], f16)
            w1_sb = load_const("w1", w1, [CH, H], f16)
            w2_sb = load_const("w2", w2, [H, H], f16)
            w3_sb = load_const("w3", w3, [H, 16], f16)
            w4_sb = load_const("w4", w4, [16, 1], f16)
            b1_sb = load_const("b1", b1, [H, 1], f32)
            b2_sb = load_const("b2", b2, [H, 1], f32)
            b3_sb = load_const("b3", b3, [16, 1], f32)
            b4_sb = load_const("b4", b4v, [1, 1], f32)

            zT_sh = pp.tile([H, SHARD], f16)
            aggS2 = pp.tile([H, SHARD], f32)
            z_shard = dp.tile([SHARD, P], f16)
            z_full = dp.tile([c.N, P], f16)  # [slice][core][row][ch]

            relu = mybir.ActivationFunctionType.Relu
            qrr = [0]  # round-robin SWDGE queue
            state = {}

            def tile_cols(t):
                return min(P, SHARD - t * P)

            def gather_unit(lp, u, pk_base, gbuf, table, t0, t1, elem):
                sm = state["sm"]
                icols = u["tot"] // 16
                nch = u["nch"]
                nb = len(u["bnd"])
                pk_sb = sm.tile([P, icols + nch + nb], i16, tag="pk")
                nc.sync.dma_start(
                    out=pk_sb[:, :icols],
                    in_=pk[:, pk_base + u["igo"] : pk_base + u["igo"] + icols],
                )
                nc.sync.dma_start(
                    out=pk_sb[:, icols : icols + nch],
                    in_=pk[
                        :,
                        pk_base + lp.IGOT + u["cgo"] :
                        pk_base + lp.IGOT + u["cgo"] + nch,
                    ],
                )
                if nb:
                    nc.sync.dma_start(
                        out=pk_sb[:, icols + nch :],
                        in_=pk[
                            :,
                            pk_base + lp.IGOT + lp.CGOT + u["bgo"] :
                            pk_base + lp.IGOT + lp.CGOT + u["bgo"] + nb,
                        ],
                    )
                for s0 in range(0, u["tot"], 1024):
                    n = min(1024, u["tot"] - s0)
                    nc.gpsimd.dma_gather(
                        gbuf[:, s0 // P : (s0 + n) // P, :],
                        table[t0:t1, :],
                        pk_sb[:, s0 // 16 : (s0 + n) // 16],
                        n, n, elem,
                        queue_num=qrr[0] % 4,
                    )
                    qrr[0] += 1
                return (pk_sb[:, icols : icols + nch].bitcast(f16),
                        pk_sb[:, icols + nch :].bitcast(f16) if nb else None)

            def build_oh(u, dl_sb, bnd_sb, ohp, tag):
                nch = u["nch"]
                oh = ohp.tile([P, nch, P], f16, tag=tag, name="oh" + tag)
                nc.vector.tensor_tensor(
                    out=oh[:],
                    in0=dl_sb.unsqueeze(2).to_broadcast([P, nch, P]),
                    in1=iota_sb[:].unsqueeze(1).to_broadcast([P, nch, P]),
                    op=mybir.AluOpType.is_equal,
                )
                ohn = None
                nb = len(u["bnd"])
                if nb:
                    ohn = ohp.tile([P, nb, P], f16, tag=tag + "n",
                                   name="ohn" + tag)
                    nc.vector.tensor_tensor(
                        out=ohn[:],
                        in0=bnd_sb.unsqueeze(2).to_broadcast([P, nb, P]),
                        in1=iota2_sb[:].unsqueeze(1).to_broadcast([P, nb, P]),
                        op=mybir.AluOpType.is_equal,
                    )
                return oh, ohn

            # ---------------- phase 1 (batch-major) ----------------
            with (
                tc.tile_pool(name="l1g", bufs=10) as gp,
                tc.tile_pool(name="l1oh", bufs=6) as ohp,
                tc.tile_pool(name="l1sm", bufs=6) as sm,
                tc.tile_pool(name="l1ps", bufs=c.T_B + 1, space="PSUM") as psa,
                tc.tile_pool(name="l1pst", bufs=1, space="PSUM") as pst,
            ):
                state["sm"] = sm
                ag_row = [0]

                def issue_ag(r1):
                    r1 = min(r1, SHARD)
                    while ag_row[0] + c.SROW <= r1:
                        r0 = ag_row[0]
                        rs = r0 + c.SROW
                        nc.gpsimd.collective_compute(
                            "AllGather",
                            mybir.AluOpType.bypass,
                            replica_groups=[list(range(c.NCORE))],
                            ins=[z_shard[r0:rs, :].opt()],
                            outs=[
                                z_full[r0 * c.NCORE : rs * c.NCORE, :].opt()
                            ],
                        )
                        ag_row[0] = rs

                for b in range(c.NB):
                    units = [L1.units[b * c.NQ + q] for q in range(c.NQ)]
                    assert all(u["b"] == b for u in units)
                    gbufs, ohs, ohns = [], [], []
                    for q, u in enumerate(units):
                        gb = gp.tile([P, u["nch"], CH], f16, tag="g",
                                     name=f"g1_{q}")
                        dl_sb, bnd_sb = gather_unit(
                            L1, u, 0, gb, x16, q * QS, (q + 1) * QS, CH)
                        oh, ohn = build_oh(u, dl_sb, bnd_sb, ohp, "o")
                        gbufs.append(gb)
                        ohs.append(oh)
                        ohns.append(ohn)
                    # piece list across the batch's 4 units
                    mm = []
                    for q, u in enumerate(units):
                        bnd_ix = {ch: j for j, ch in enumerate(u["bnd"])}
                        for (ch, t, var) in u["pieces"]:
                            mm.append((q, ch, t, var, bnd_ix.get(ch)))
                    first, last, aggs = {}, {}, {}
                    for i, (q, ch, t, var, j) in enumerate(mm):
                        first.setdefault(t, i)
                        last[t] = i
                    for t in units[0]["tiles"]:
                        aggs[t] = psa.tile([CH, P], f32, tag="agg1",
                                           name=f"agg1_{t}")
                    for i, (q, ch, t, var, j) in enumerate(mm):
                        rhs = (ohs[q][:, ch : ch + 1, :] if var == 0
                               else ohns[q][:, j : j + 1, :])
                        nc.tensor.matmul(
                            out=aggs[t][:],
                            lhsT=gbufs[q][:, ch : ch + 1, :],
                            rhs=rhs,
                            start=(i == first[t]),
                            stop=(i == last[t]),
                        )
                    for t in units[0]["tiles"]:
                        tw = tile_cols(t)
                        xT_sb = sm.tile([CH, P], f32, tag="xt", name="xt")
                        nc.scalar.dma_start(
                            out=xT_sb[:, :tw], in_=xT[:, t * P : t * P + tw]
                        )
                        sT = sm.tile([CH, P], f16, tag="st", name="st")
                        nc.vector.tensor_add(
                            out=sT[:, :tw], in0=aggs[t][:, :tw],
                            in1=xT_sb[:, :tw],
                        )
                        h1p = pst.tile([H, P], f32, tag="h1", name="h1p")
                        nc.tensor.matmul(
                            out=h1p[:, :tw], lhsT=w1_sb[:], rhs=sT[:, :tw],
                            start=True, stop=True,
                        )
                        h1f = sm.tile([H, P], f16, tag="h1f", name="h1f")
                        nc.scalar.activation(
                            out=h1f[:, :tw], in_=h1p[:, :tw], func=relu,
                            bias=b1_sb[:],
                        )
                        zp = pst.tile([H, P], f32, tag="zp", name="zp")
                        nc.tensor.matmul(
                            out=zp[:, :tw], lhsT=w2_sb[:], rhs=h1f[:, :tw],
                            start=True, stop=True,
                        )
                        nc.vector.tensor_copy(
                            out=zT_sh[:, t * P : t * P + tw], in_=zp[:, :tw]
                        )
                        zf = sm.tile([H, P], f16, tag="zf", name="zf")
                        nc.vector.tensor_copy(out=zf[:, :tw], in_=zp[:, :tw])
                        ztr = pst.tile([P, H], f16, tag="ztr", name="ztr")
                        nc.tensor.transpose(
                            out=ztr[:tw, :], in_=zf[:, :tw], identity=id_sb[:]
                        )
                        zr = sm.tile([P, P], f16, tag="zr", name="zr")
                        nc.vector.memset(zr[:, H:], 0)
                        nc.vector.tensor_copy(out=zr[:tw, :H], in_=ztr[:tw, :])
                        nc.scalar.dma_start(
                            out=z_shard[t * P : t * P + tw, :], in_=zr[:tw, :]
                        )
                    done_rows = (units[0]["tiles"][-1] + 1) * P
                    issue_ag((done_rows // c.SROW) * c.SROW)
                issue_ag(SHARD)

            # ---------------- phase 2 (group-major) ----------------
            pk2 = L1.IGOT + L1.CGOT + L1.BGOT
            with (
                tc.tile_pool(name="l2g", bufs=6) as gp,
                tc.tile_pool(name="l2oh", bufs=6) as ohp,
                tc.tile_pool(name="l2sm", bufs=6) as sm,
                tc.tile_pool(name="l2ps", bufs=2, space="PSUM") as psa,
                tc.tile_pool(name="l2pst", bufs=1, space="PSUM") as pst,
            ):
                state["sm"] = sm
                SR8 = c.SROW * c.NCORE
                for ui, u in enumerate(L2.units):
                    g = u["grp"]
                    assert ui // c.NB == g
                    g2 = gp.tile([P, u["nch"], P], f16, tag="g2", name="g2")
                    dl_sb, bnd_sb = gather_unit(
                        L2, u, pk2, g2, z_full, g * SR8, (g + 1) * SR8, P)
                    oh, ohn = build_oh(u, dl_sb, bnd_sb, ohp, "o2")
                    bnd_ix = {ch: j for j, ch in enumerate(u["bnd"])}
                    first, last = {}, {}
                    for i, (ch, t, var) in enumerate(u["pieces"]):
                        first.setdefault(t, i)
                        last[t] = i
                    tiles = u["tiles"]
                    t0b = tiles[0]
                    slab = psa.tile([H, len(tiles) * P], f32, tag="agg2",
                                    name="agg2")
                    for i, (ch, t, var) in enumerate(u["pieces"]):
                        rhs = (oh[:, ch : ch + 1, :] if var == 0
                               else ohn[:, bnd_ix[ch] : bnd_ix[ch] + 1, :])
                        nc.tensor.matmul(
                            out=slab[:, (t - t0b) * P : (t - t0b + 1) * P],
                            rhs=rhs,
                            lhsT=g2[:, ch : ch + 1, :H],
                            start=(i == first[t]),
                            stop=(i == last[t]),
                        )
                    bw = tiles[-1] * P + tile_cols(tiles[-1]) - t0b * P
                    bcols = slice(t0b * P, t0b * P + bw)
                    if g == 0:
                        nc.vector.tensor_copy(
                            out=aggS2[:, bcols], in_=slab[:, :bw]
                        )
                    else:
                        nc.vector.tensor_add(
                            out=aggS2[:, bcols], in0=aggS2[:, bcols],
                            in1=slab[:, :bw],
                        )
                    for t in u["tiles"]:
                        tw = tile_cols(t)
                        cols = slice(t * P, t * P + tw)
                        if g == c.NS - 1:
                            t2 = sm.tile([H, P], f32, tag="t2", name="t2")
                            nc.vector.tensor_add(
                                out=t2[:, :tw], in0=aggS2[:, cols],
                                in1=zT_sh[:, cols],
                            )
                            h2f = sm.tile([H, P], f16, tag="h2f", name="h2f")
                            nc.scalar.activation(
                                out=h2f[:, :tw], in_=t2[:, :tw], func=relu,
                                bias=b2_sb[:],
                            )
                            h3p = pst.tile([16, P], f32, tag="h3", name="h3p")
                            nc.tensor.matmul(
                                out=h3p[:, :tw], lhsT=w3_sb[:],
                                rhs=h2f[:, :tw], start=True, stop=True,
                            )
                            h3f = sm.tile([16, P], f16, tag="h3f", name="h3f")
                            nc.scalar.activation(
                                out=h3f[:, :tw], in_=h3p[:, :tw], func=relu,
                                bias=b3_sb[:],
                            )
                            op_ = pst.tile([1, P], f32, tag="op", name="op_")
                            nc.tensor.matmul(
                                out=op_[:, :tw], lhsT=w4_sb[:],
                                rhs=h3f[:, :tw], start=True, stop=True,
                            )
                            ot = sm.tile([1, P], f32, tag="ot", name="ot")
                            nc.vector.scalar_tensor_tensor(
                                out=ot[:, :tw],
                                in0=op_[:, :tw],
                                scalar=1.0,
                                in1=b4_sb[:].to_broadcast([1, tw]),
                                op0=mybir.AluOpType.mult,
                                op1=mybir.AluOpType.add,
                            )
                            nc.scalar.dma_start(
                                out=out[:, t * P : t * P + tw],
                                in_=ot[:, :tw],
                            )
    nc.compile()
    return nc


def _in_maps(cfg, plan, x, W1, b1, W2, b2, W3, b3, W4, b4):
    c = cfg
    iota_a = np.broadcast_to(
        np.arange(P, dtype=np.float16), (P, P)).copy()
    common = dict(
        x16=x.astype(np.float16),
        iota=iota_a,
        iota2=iota_a + np.float16(128.0),
        ident=np.eye(c.H, dtype=np.float16),
        w1=W1.astype(np.float16),
        w2=W2.astype(np.float16),
        w3=W3.astype(np.float16),
        w4=W4.astype(np.float16),
        b1=b1.reshape(-1, 1).astype(np.float32),
        b2=b2.reshape(-1, 1).astype(np.float32),
        b3=b3.reshape(-1, 1).astype(np.float32),
        b4v=b4.reshape(1, 1).astype(np.float32),
    )
    in_maps = []
    for ci in range(c.NCORE):
        pk_a = plan.core_arrays(ci)
        xT_a = np.ascontiguousarray(
            x[ci * c.SHARD : (ci + 1) * c.SHARD].T.astype(np.float32)
        )
        in_maps.append(dict(common, pk=pk_a, xT=xT_a))
    return in_maps


def _run(cfg, plan, nc, x, W1, b1, W2, b2, W3, b3, W4, b4, **kw):
    from concourse.bass_utils import run_bass_kernel_spmd

    c = cfg
    in_maps = _in_maps(cfg, plan, x, W1, b1, W2, b2, W3, b3, W4, b4)
    res = run_bass_kernel_spmd(nc, in_maps, core_ids=list(range(c.NCORE)), **kw)
    outs = [res.results[ci]["out"].reshape(-1) for ci in range(c.NCORE)]
    return np.concatenate(outs).reshape(-1, 1).astype(np.float32), res


def kernel(x, edge_index, W1, b1, W2, b2, W3, b3, W4, b4):
    cfg = FULL
    x = np.asarray(x, dtype=np.float32)
    src = np.asarray(edge_index[0], dtype=np.int64)
    dst = np.asarray(edge_index[1], dtype=np.int64)
    plan = Plan(cfg, src, dst)
    nc = _build(plan)
    out, _ = _run(
        cfg, plan, nc, x,
        np.asarray(W1), np.asarray(b1), np.asarray(W2), np.asarray(b2),
        np.asarray(W3), np.asarray(b3), np.asarray(W4), np.asarray(b4),
    )
    return out
